# revision 1
# baseline (speedup 1.0000x reference)
"""GATConv Trainium kernel builder (single-core SPMD program) + host prep.

Per-core program (identical NEFF on all 8 cores, different input data):
  Node tables are ROTATED per core: table row r = global node
  (dev_base + r) % N, so every core's own nodes are rows 0..DEV_N-1 and the
  program stays core-independent. The host rotates xT and all indices.

  Phase 1 (all V rows): h_ext[r, 0:132] = [x@W.T | a_src] (f32r, 192-wide
  rows for dma_gather's 256B-multiple elem constraint; cols 132:192 unwritten
  junk, never read). a_dev[r, 0:4] = a_dst for own rows r < DEV_N (64-wide
  rows, junk beyond col 4).

  Phase 2, per dst-block (128 own nodes), edges pre-routed/sorted by host:
  - dma_gather h_ext rows by src (int16 idxs => lo section: src < 32768 from
    h_ext[0:], hi section: src-32768 from h_ext[32768:]) -> stage tile.
  - dma_gather a_dev rows by local dst -> agath tile (one per block).
  - ea = exp(leaky_relu(a_src[src] + a_dst[dst])), Gs = h[src]*ea.
  - rhs tile per edge-tile j: [Gs(128) | ea(4) | h|a_src(132)]; one-hot
    sel[e, m] = (dst_loc[e] == m); PSUM accumulates sel.T @ rhs over the
    block's tiles => [P | s | Q | junk].
  - out = P/s + Q.

Edge layout: per block, lo-section edges then hi-section edges, each padded
to global fixed tile counts (T_LO / T_HI) with idx-0 edges carrying
dst_loc = -1 (zero one-hot row => no contribution). Edge i of a section is
at (lane = i%128, tile = i//128); dma_gather's index j lives at
idx16[j%16, j//16], replicated 8x down the 128 partitions.
"""

import numpy as np

import concourse.bass as bass
import concourse.bacc as bacc
import concourse.mybir as mybir
import concourse.tile as tile
from concourse import library_config

DT = mybir.dt
ALU = mybir.AluOpType
ACTF = mybir.ActivationFunctionType

F = 128    # feature dim (in == out)
NH = 4     # heads
HD = 32    # head dim
HEC = 132  # used h_ext cols: h(128) | a_src(4)
GE = 192   # h_ext gather elem width (f32 -> 768B, mult of 256B)
AE = 64    # a_dev row width (256B)
RC = 260   # rhs per-tile block: Gs(128) | ea(4) | h(128)
UNIT = 12  # tiles per pipeline unit


def build_gat_nc(V, DEV_N, T_LO, T_HI, HALF=32768, leaky=0.2):
    """Build the single-core Bass program."""
    T = T_LO + T_HI
    NBLK = (DEV_N + 127) // 128
    NT = NBLK * T

    nc = bacc.Bacc(num_swdge_queues=4)
    xT = nc.declare_dram_parameter("xT", [F, V], DT.float32r, isOutput=False)
    Wnat = nc.declare_dram_parameter("Wnat", [F, F], DT.float32, isOutput=False)
    Wt = nc.declare_dram_parameter("Wt", [F, F], DT.float32r, isOutput=False)
    Aatt = nc.declare_dram_parameter("Aatt", [F, 2 * NH], DT.float32,
                                     isOutput=False)
    gidx = nc.declare_dram_parameter("gidx", [128, NT * 8], DT.int16,
                                     isOutput=False)
    dstLb = nc.declare_dram_parameter("dstLb", [128, NT * 128], DT.int16,
                                      isOutput=False)
    dstL = nc.declare_dram_parameter("dstL", [128, NT], DT.int32,
                                     isOutput=False)
    NU = -(-T_LO // UNIT) + -(-T_HI // UNIT)
    vcnt = nc.declare_dram_parameter("vcnt", [128, NBLK * NU], DT.int32,
                                     isOutput=False)
    out = nc.declare_dram_parameter("out", [DEV_N, F], DT.float32,
                                    isOutput=True)

    h_ext = nc.dram_tensor("h_ext", [V, GE], DT.float32r)
    a_dev = nc.dram_tensor("a_dev", [NBLK * 128, AE], DT.float32r)

    with tile.TileContext(nc) as tc:
        with (
            tc.tile_pool(name="const", bufs=1) as const,
            tc.tile_pool(name="p1", bufs=3) as p1,
            tc.tile_pool(name="p1ps", bufs=2, space="PSUM") as p1ps,
            tc.tile_pool(name="p2", bufs=2) as p2,
            tc.tile_pool(name="pu", bufs=4) as pu,
            tc.tile_pool(name="p2ps", bufs=2, space="PSUM") as p2ps,
        ):
            nc.gpsimd.load_library(library_config.mlp)

            # ---- constants ----
            wext = const.tile([128, 256], DT.float32r)
            zero_c = const.tile([128, 1], DT.float32)
            nc.gpsimd.memset(zero_c[:], 0.0)
            nc.vector.tensor_copy(
                out=wext[:, F + 2 * NH:256],
                in_=bass.AP(zero_c[:].tensor, 0,
                            [[1, 128], [0, 256 - F - 2 * NH]]))
            wnat_t = const.tile([128, F], DT.float32)
            aatt_t = const.tile([128, 2 * NH], DT.float32)
            iota_t = const.tile([128, 128], DT.int32)
            iota_c = const.tile([128, 1], DT.float32)
            leak_c = const.tile([128, 1], DT.float32)
            nc.gpsimd.iota(iota_c[:], pattern=[[0, 1]], base=0,
                           channel_multiplier=1,
                           allow_small_or_imprecise_dtypes=True)
            nc.gpsimd.memset(leak_c[:], leaky)
            nc.sync.dma_start(out=wnat_t[:], in_=Wnat[:, :])
            nc.sync.dma_start(out=aatt_t[:], in_=Aatt[:, :])
            nc.sync.dma_start(out=wext[:, 0:F], in_=Wt[:, :])
            nc.gpsimd.iota(iota_t[:], pattern=[[1, 128]], base=0,
                           channel_multiplier=0)
            vps = p1ps.tile([128, 2 * NH], DT.float32, tag="vps")
            nc.tensor.matmul(out=vps[:], lhsT=wnat_t[:], rhs=aatt_t[:],
                             start=True, stop=True)
            nc.vector.tensor_copy(out=wext[:, F:F + 2 * NH], in_=vps[:])

            # ---- phase 1 (batches of 8 node chunks) ----
            nchunks = (V + 127) // 128
            CBATCH = 8
            for cb in range(0, nchunks, CBATCH):
                nb = min(CBATCH, nchunks - cb)
                c0 = cb * 128
                nn = min(V - c0, nb * 128)
                xc = p1.tile([128, CBATCH * 128], DT.float32r, tag="xc")
                nc.scalar.dma_start(out=xc[:, :nn], in_=xT[:, c0:c0 + nn])
                hrow = p1.tile([128, CBATCH * HEC], DT.float32r, tag="hrow")
                arow = p1.tile([128, CBATCH * NH], DT.float32r, tag="arow")
                for k in range(nb):
                    m = min(128, V - (c0 + k * 128))
                    hps = p1ps.tile([128, 256], DT.float32, tag="hps")
                    nc.tensor.matmul(
                        out=hps[:m, :],
                        lhsT=xc[:, k * 128:k * 128 + m],
                        rhs=wext[:],
                        start=True, stop=True)
                    nc.vector.tensor_copy(
                        out=hrow[:m, k * HEC:(k + 1) * HEC],
                        in_=hps[:m, 0:HEC])
                    nc.vector.tensor_copy(
                        out=arow[:m, k * NH:(k + 1) * NH],
                        in_=hps[:m, HEC:HEC + NH])
                # strided batched writes: table row c0 + k*128 + p
                last = min(V, c0 + nb * 128)
                kfull = (last - c0) // 128  # full 128-row chunks in batch
                if kfull > 0:
                    nc.sync.dma_start(
                        out=bass.AP(h_ext[:, :].tensor, c0 * GE,
                                    [[GE, 128], [GE * 128, kfull], [1, HEC]]),
                        in_=hrow[:].rearrange("p (k c) -> p k c", c=HEC)[
                            :, 0:kfull, :])
                for k in range(kfull, nb):
                    m = min(128, V - (c0 + k * 128))
                    nc.sync.dma_start(
                        out=h_ext[c0 + k * 128:c0 + k * 128 + m, 0:HEC],
                        in_=hrow[:m, k * HEC:(k + 1) * HEC])
                if c0 < DEV_N:
                    ka = min(kfull, max(0, (DEV_N - c0) // 128))
                    if ka > 0:
                        nc.sync.dma_start(
                            out=bass.AP(a_dev[:, :].tensor, c0 * AE,
                                        [[AE, 128], [AE * 128, ka], [1, NH]]),
                            in_=arow[:].rearrange("p (k c) -> p k c", c=NH)[
                                :, 0:ka, :])
                    for k in range(ka, nb):
                        ck0 = c0 + k * 128
                        if ck0 >= DEV_N:
                            break
                        mm = min(128, DEV_N - ck0)
                        nc.sync.dma_start(
                            out=a_dev[ck0:ck0 + mm, 0:NH],
                            in_=arow[:mm, k * NH:(k + 1) * NH])

            # ---- phase 2 ----
            for b in range(NBLK):
                rows = min(128, DEV_N - b * 128)
                dl = p2.tile([128, T], DT.int32, tag="dl")
                nc.sync.dma_start(out=dl[:], in_=dstL[:, b * T:(b + 1) * T])
                gi = p2.tile([128, T * 8], DT.int16, tag="gi")
                nc.sync.dma_start(out=gi[:],
                                  in_=gidx[:, b * T * 8:(b + 1) * T * 8])
                a_blk = p2.tile([128, NH], DT.float32r, tag="a_blk")
                nc.sync.dma_start(out=a_blk[:],
                                  in_=a_dev[b * 128:(b + 1) * 128, 0:NH])
                pa = p2ps.tile([128, T * NH], DT.float32, tag="pa")
                par = pa[:].rearrange("p (t e) -> p t e", e=NH)

                acc = p2ps.tile([128, RC], DT.float32, tag="acc")
                sections = [(0, T_LO, 0)]
                if T_HI > 0:
                    sections.append((T_LO, T_HI, HALF))
                units = []
                for t0, Ts, roff in sections:
                    for u in range(0, Ts, UNIT):
                        units.append((t0 + u, min(UNIT, Ts - u), roff))
                for ui, (t0, Tu, roff) in enumerate(units):
                    stage = pu.tile([128, UNIT * GE], DT.float32r,
                                    tag="stage")
                    sr = stage[:].rearrange("p (t g) -> p t g", g=GE)
                    nc.gpsimd.dma_gather(
                        out_ap=sr[:, 0:Tu, :],
                        in_ap=h_ext[roff:, :],
                        idxs_ap=gi[:, t0 * 8:(t0 + Tu) * 8],
                        num_idxs=Tu * 128, num_idxs_reg=Tu * 128,
                        elem_size=GE, single_packet=False,
                        queue_num=2 if ui % 2 == 0 else 3)

                    rhs = pu.tile([128, UNIT * RC], DT.float32r, tag="rhs")
                    rr = rhs[:].rearrange("p (t c) -> p t c", c=RC)

                    # transposed one-hot selT[m, (t,e)] = (dstL[e,t] == m)
                    selT = pu.tile([128, UNIT * 128], DT.float32r, tag="selT")
                    selTr = selT[:].rearrange("p (t e) -> p t e", e=128)
                    lb0 = (b * T + t0) * 128
                    dlb = pu.tile([128, UNIT * 128], DT.int16, tag="dlb")
                    nc.sync.dma_start(out=dlb[:, 0:Tu * 128],
                                      in_=dstLb[:, lb0:lb0 + Tu * 128])
                    nc.vector.tensor_tensor(
                        out=selTr[:, 0:Tu, :],
                        in0=dlb[:, 0:Tu * 128].rearrange(
                            "p (t e) -> p t e", e=128),
                        in1=bass.AP(iota_c[:].tensor, 0,
                                    [[1, 128], [0, Tu], [0, 128]]),
                        op=ALU.is_equal)
                    for j in range(Tu):
                        nc.tensor.matmul(
                            out=par[:, t0 + j, :], lhsT=selTr[:, j, :],
                            rhs=a_blk[:], start=True, stop=True)

                    # ea chain: alpha -> leaky -> exp into rhs[:, :, 128:132]
                    scr = pu.tile([128, UNIT * NH], DT.float32, tag="scr")
                    scrr = scr[:].rearrange("p (t e) -> p t e", e=NH)
                    nc.vector.tensor_tensor(
                        out=scrr[:, 0:Tu, :], in0=sr[:, 0:Tu, F:F + NH],
                        in1=par[:, t0:t0 + Tu, :], op=ALU.add)
                    scr2 = pu.tile([128, UNIT * NH], DT.float32, tag="scr2")
                    scr2r = scr2[:].rearrange("p (t e) -> p t e", e=NH)
                    nc.vector.tensor_tensor(
                        out=scr2r[:, 0:Tu, :], in0=scrr[:, 0:Tu, :],
                        in1=bass.AP(leak_c[:].tensor, 0,
                                    [[1, 128], [0, Tu], [0, NH]]),
                        op=ALU.mult)
                    nc.vector.tensor_tensor(
                        out=scrr[:, 0:Tu, :], in0=scrr[:, 0:Tu, :],
                        in1=scr2r[:, 0:Tu, :], op=ALU.max)
                    nc.scalar.activation(out=rr[:, 0:Tu, F:F + NH],
                                         in_=scrr[:, 0:Tu, :], func=ACTF.Exp)

                    # h copy on scalar engine
                    nc.scalar.copy(out=rr[:, 0:Tu, HEC:RC],
                                   in_=sr[:, 0:Tu, 0:F])
                    # Gs = h * ea (per-head broadcast)
                    nc.vector.tensor_tensor(
                        out=rr[:, 0:Tu, 0:F].rearrange(
                            "p t (h e) -> p t h e", e=HD),
                        in0=sr[:, 0:Tu, 0:F].rearrange(
                            "p t (h e) -> p t h e", e=HD),
                        in1=rr[:, 0:Tu, F:F + NH][:, :, :, None].to_broadcast(
                            [128, Tu, NH, HD]),
                        op=ALU.mult)

                    # one-hot + accumulate
                    sel = pu.tile([128, UNIT * 128], DT.float32r, tag="sel")
                    selr = sel[:].rearrange("p (t m) -> p t m", m=128)
                    nc.vector.tensor_tensor(
                        out=selr[:, 0:Tu, :],
                        in0=dl[:, t0:t0 + Tu][:, :, None].to_broadcast(
                            [128, Tu, 128]),
                        in1=iota_t[:][:, None, :].to_broadcast([128, Tu, 128]),
                        op=ALU.is_equal)
                    for j in range(Tu):
                        nc.tensor.matmul(
                            out=acc[:], lhsT=selr[:, j, :], rhs=rr[:, j, :],
                            start=(ui == 0 and j == 0),
                            stop=(ui == len(units) - 1 and j == Tu - 1))

                # ---- evac: out = P / s + Q ----
                sden = p2.tile([128, NH], DT.float32, tag="sden")
                nc.vector.tensor_scalar_max(out=sden[:], in0=acc[:, F:F + NH],
                                            scalar1=1e-30)
                rs = p2.tile([128, NH], DT.float32, tag="rs")
                nc.vector.reciprocal(out=rs[:], in_=sden[:])
                ot = p2.tile([128, F], DT.float32, tag="ot")
                otr = ot[:].rearrange("p (h e) -> p h e", e=HD)
                nc.vector.tensor_tensor(
                    out=otr,
                    in0=acc[:, 0:F].rearrange("p (h e) -> p h e", e=HD),
                    in1=rs[:][:, :, None].to_broadcast([128, NH, HD]),
                    op=ALU.mult)
                nc.vector.tensor_tensor(
                    out=otr, in0=otr,
                    in1=acc[:, HEC:HEC + F].rearrange("p (h e) -> p h e", e=HD),
                    op=ALU.add)
                nc.sync.dma_start(out=out[b * 128:b * 128 + rows, :],
                                  in_=ot[:rows, :])

    return nc


def route_edges(edge_index, N, n_cores, half=32768):
    """Host edge routing. Returns (T_LO, T_HI, per_core index dicts)."""
    src = np.concatenate([np.asarray(edge_index[0]),
                          np.arange(N)]).astype(np.int64)
    dst = np.concatenate([np.asarray(edge_index[1]),
                          np.arange(N)]).astype(np.int64)
    dev_n = N // n_cores
    assert dev_n * n_cores == N
    core = dst // dev_n
    nblk = (dev_n + 127) // 128

    per_core_raw = []
    T_LO = T_HI = 0
    for d in range(n_cores):
        m = core == d
        s_rot = (src[m] - d * dev_n) % N
        d_loc = dst[m] - d * dev_n
        blk = d_loc // 128
        lo = s_rot < half
        cnt_lo = np.bincount(blk[lo], minlength=nblk)
        cnt_hi = np.bincount(blk[~lo], minlength=nblk)
        T_LO = max(T_LO, int(-(-cnt_lo.max() // 128)))
        T_HI = max(T_HI, int(-(-cnt_hi.max() // 128)))
        per_core_raw.append((s_rot, d_loc, blk, lo))
    T_HI = max(T_HI, 1)
    T = T_LO + T_HI

    per_core = []
    NT = nblk * T
    for d in range(n_cores):
        s_rot, d_loc, blk, lo = per_core_raw[d]
        gidx16 = np.zeros((16, NT * 8), dtype=np.int16)
        dstL = np.full((128, NT), -1, dtype=np.int32)
        nu_lo = -(-T_LO // 12)
        nu_hi = -(-T_HI // 12)
        unit_sizes = ([min(12, T_LO - u) * 128 for u in range(0, T_LO, 12)] +
                      [min(12, T_HI - u) * 128 for u in range(0, T_HI, 12)])
        vcnt_c = np.tile(np.array(unit_sizes, dtype=np.int32), nblk)
        for b in range(nblk):
            bcol = b * T * 8
            for sec in (0, 1):
                if sec == 0:
                    bm = (blk == b) & lo
                    vals = s_rot[bm]
                    t0, sec_col = 0, bcol
                else:
                    bm = (blk == b) & ~lo
                    vals = s_rot[bm] - half
                    t0, sec_col = T_LO, bcol + T_LO * 8
                n = len(vals)
                if n == 0:
                    continue
                jj = np.arange(n)
                gidx16[jj % 16, sec_col + jj // 16] = vals.astype(np.int16)
                dstL[jj % 128, b * T + t0 + jj // 128] = d_loc[bm] - b * 128

        # windows with zero real edges got a synthetic idx-0 entry above;
        # nothing else needed (their dstL stays -1).
        vcnt_b = np.ascontiguousarray(np.broadcast_to(
            vcnt_c[None, :], (128, len(vcnt_c))).astype(np.int32))
        dstLb = np.ascontiguousarray(np.broadcast_to(
            dstL.T.reshape(1, -1), (128, NT * 128)).astype(np.int16))
        per_core.append({
            "gidx": np.tile(gidx16, (8, 1)),
            "dstLb": dstLb,
            "dstL": dstL,
            "vcnt": vcnt_b,
        })
    return T_LO, T_HI, per_core


def host_prep(x, edge_index, W, att_src, att_dst, n_cores, half=32768):
    """Returns (T_LO, T_HI, per-core in_maps list)."""
    N = x.shape[0]
    dev_n = N // n_cores
    xTf = np.ascontiguousarray(np.asarray(x).T.astype(np.float32))
    Wnat = np.ascontiguousarray(np.asarray(W).astype(np.float32))
    Wt = np.ascontiguousarray(Wnat.T)
    A = np.zeros((F, 2 * NH), dtype=np.float32)
    for h in range(NH):
        A[h * HD:(h + 1) * HD, h] = np.asarray(att_src)[0, h]
        A[h * HD:(h + 1) * HD, NH + h] = np.asarray(att_dst)[0, h]
    T_LO, T_HI, per_core = route_edges(edge_index, N, n_cores, half)
    in_maps = []
    for d in range(n_cores):
        xr = np.roll(xTf, -d * dev_n, axis=1)
        in_maps.append(dict(per_core[d], xT=np.ascontiguousarray(xr),
                            Wnat=Wnat, Wt=Wt, Aatt=A))
    return T_LO, T_HI, in_maps


# ---------------------------------------------------------------------------
# Self-contained kernel entry point (full problem size hardcoded).
# ---------------------------------------------------------------------------
N_NODES = 50000
N_CORES = 8
HALF_SPLIT = 32768


def _run(inputs, trace=False):
    import time
    from concourse.bass_utils import run_bass_kernel_spmd

    x = np.asarray(inputs["x"], dtype=np.float32)
    edge_index = np.asarray(inputs["edge_index"])
    W = np.asarray(inputs["W"], dtype=np.float32)
    att_src = np.asarray(inputs["att_src"], dtype=np.float32)
    att_dst = np.asarray(inputs["att_dst"], dtype=np.float32)

    N = x.shape[0]
    assert N == N_NODES, N
    dev_n = N // N_CORES

    t0 = time.time()
    T_LO, T_HI, in_maps = host_prep(x, edge_index, W, att_src, att_dst,
                                    N_CORES, half=HALF_SPLIT)
    t1 = time.time()
    nc = build_gat_nc(N, dev_n, T_LO, T_HI, HALF=HALF_SPLIT)
    nc.compile()
    t2 = time.time()
    res = run_bass_kernel_spmd(nc, in_maps, list(range(N_CORES)), trace=trace)
    t3 = time.time()
    print(f"kernel: host_prep {t1-t0:.1f}s build+compile {t2-t1:.1f}s "
          f"run {t3-t2:.1f}s T_LO={T_LO} T_HI={T_HI}")
    out = np.concatenate([res.results[d]["out"] for d in range(N_CORES)],
                         axis=0).astype(np.float32)
    return out, res.exec_time_ns


def kernel(**inputs) -> np.ndarray:
    return _run(inputs, trace=False)[0]

